# revision 8
# baseline (speedup 1.0000x reference)
"""Varlen causal sliding-window attention with per-head sink logits, on 8 trn2 cores.

Wall time under the axon tunnel is transfer-bound (~20 ms/MiB each way, plus
~34 ms per tensor), so the wire format is ONE int8 tensor per core holding
int8-quantized q / k|v (per-(token,head) scales) plus a trailing row-block with
the f32 scales bitcast to bytes; the output is int8 with per-channel scales.
All dequant/requant runs on-device where compute is ~free (cost-model exec
~0.3 ms vs ~1.4 s of transfer per call).

Sharding: data-parallel over (batch, head-group). Each core gets one batch's
tokens and 16/PB contiguous q-heads (PB = 8//B parts per batch) plus the
matching kv-heads.

Device input (per core), comb [S+128, PW] int8 with PW = HL*128 + KVL*256:
  rows [0,S):   cols [0, HL*128) = q natural int8;
                cols [HL*128, PW) = per local kv head, 128 k-cols | 128 v-cols
  rows [S,S+128): bitcast f32 aux row-block, value columns
                qs [HL*NT] | sks [KVL*NT] | vs [KVL*NT] | sinks [HL]
                (aux[p, h*NT+t] = scale of token t*128+p, head h; sks
                premultiplied by softmax SCALE)
Device output:
  o [HL*128, S+4] int8 (O^T layout); the last 4 bytes of each row are the
  bitcast f32 per-channel absmax m (host dequant is o[:, :S] * m/127).

Device kernel per head:
  - load: int8 tiles DMA'd natural-layout; q dequants per-token via DVE
    tensor_scalar (int8 x [128,1] f32 -> f16) then PE-transposes per 128-block
    into qT; k converts int8->f16 (scales NOT applied) and PE-transposes into
    kT; v dequants in place (lhsT consumed natural).
  - QK^T per 128-key tile into PSUM; exp evicts PSUM->SBUF f16 probs with the
    per-key scale folded into the activation's per-partition scale AP
    (logit = qdeq . k_int, exp scale = SCALE*ks_key); triangular masks fix the
    two band edges.
  - PV + ones-matmul denominator per 256-query span accumulate O^T and D in
    one PSUM bank; DVE adds exp(sink), reciprocal, multiply into an f32 O^T
    row; after all spans: rowmax(|O|), quantize row to int8, DMA out.
"""

import sys

sys.path.insert(0, "/opt/trn_rl_repo")

import numpy as np

NUM_HEADS = 16
NUM_KV_HEADS = 4
HEAD_DIM = 128
WINDOW = 1024
SCALE = 0.08838834764831845
TILE = 128

_CACHE = {}
_JIT = {}


def _band_width(kj, S):
    # keys in tile kj are visible to queries q with 0 <= q - k <= WINDOW
    # -> q in [kj*TILE, kj*TILE + WINDOW + TILE), clipped to S
    return min(S, kj * TILE + WINDOW + TILE) - kj * TILE


def _chunks(w):
    # split [0, w) at 512 boundaries (PSUM bank) for matmul outputs
    out = []
    c0 = 0
    while c0 < w:
        out.append((c0, min(512, w - c0)))
        c0 += 512
    return out


def build_nc(S, HL, KVL):
    import concourse.bacc as bacc
    import concourse.mybir as mybir
    from concourse.masks import make_identity, make_lower_triangular, make_upper_triangular
    from concourse.tile import TileContext

    f32 = mybir.dt.float32
    f16 = mybir.dt.float16
    i8 = mybir.dt.int8
    NT = S // TILE
    WMAX = min(S, WINDOW + TILE)
    SUMW = sum(_band_width(kj, S) for kj in range(NT))
    OFF = np.cumsum([0] + [_band_width(kj, S) for kj in range(NT)]).tolist()
    SPAN = 256
    NSPAN = S // SPAN
    PW = HL * TILE + KVL * 2 * TILE
    AUXW = HL * NT + 2 * KVL * NT + HL
    KOFF = HL * TILE  # comb col where the kv block starts

    nc = bacc.Bacc()
    comb_d = nc.dram_tensor("comb", [S + TILE, PW], i8, kind="ExternalInput")
    # last 4 int8 cols of each row hold the f32 per-channel absmax (bitcast)
    o_d = nc.dram_tensor("o", [HL * TILE, S + 4], i8, kind="ExternalOutput")

    with TileContext(nc) as tc:
        with (
            tc.tile_pool(name="const", bufs=1) as const_pool,
            tc.tile_pool(name="qi8", bufs=2) as qi8_pool,
            tc.tile_pool(name="kvi8", bufs=2) as kvi8_pool,
            tc.tile_pool(name="dq", bufs=3) as dq_pool,
            tc.tile_pool(name="qT", bufs=3) as qT_pool,
            tc.tile_pool(name="kT", bufs=2) as kT_pool,
            tc.tile_pool(name="vv", bufs=2) as v_pool,
            tc.tile_pool(name="pT", bufs=3) as pT_pool,
            tc.tile_pool(name="dsb", bufs=3) as d_pool,
            tc.tile_pool(name="orow", bufs=2) as orow_pool,
            tc.tile_pool(name="oi8", bufs=2) as oi8_pool,
            tc.tile_pool(name="stat", bufs=4) as stat_pool,
            tc.tile_pool(name="spsum", bufs=1, space="PSUM") as s_psum,
            tc.tile_pool(name="opsum", bufs=2, space="PSUM") as o_psum,
            tc.tile_pool(name="tpsum", bufs=2, space="PSUM") as t_psum,
        ):
            mask_diag = const_pool.tile([TILE, TILE], f16)  # valid: q >= k
            mask_win = const_pool.tile([TILE, TILE], f16)  # valid: q <= k
            make_upper_triangular(nc, mask_diag[:], val=1.0, diag=True)
            make_lower_triangular(nc, mask_win[:], val=1.0, diag=True)
            ones = const_pool.tile([TILE, TILE], f16)
            nc.vector.memset(ones[:], 1.0)
            ident = const_pool.tile([TILE, TILE], f16)
            make_identity(nc, ident[:])
            aux_i8 = const_pool.tile([TILE, AUXW * 4], i8)
            nc.sync.dma_start(out=aux_i8[:], in_=comb_d[S : S + TILE, : AUXW * 4])

            def aux_f32(a, b):
                # f32 view of aux value columns [a, b)
                return aux_i8[:, a * 4 : b * 4].bitcast(f32)

            esk = const_pool.tile([TILE, HL], f32)
            nc.scalar.activation(
                esk[:],
                aux_f32(HL * NT + 2 * KVL * NT, AUXW),
                mybir.ActivationFunctionType.Exp,
            )

            kT_sb = None
            v_by_kv = {}
            pT_by_hl = {}

            def qk_phase(hl):
                nonlocal kT_sb
                kv = hl // 4 if HL >= 4 else 0
                if hl % 4 == 0 or kT_sb is None:
                    # ---- K: int8 natural -> f16 (unscaled) -> kT via PE ----
                    ki8 = kvi8_pool.tile([TILE, NT * TILE], i8, tag="ki8")
                    nc.sync.dma_start(
                        out=ki8[:].rearrange("p (t d) -> p t d", d=TILE),
                        in_=comb_d[
                            :S, KOFF + kv * 2 * TILE : KOFF + kv * 2 * TILE + TILE
                        ].rearrange("(t p) d -> p t d", p=TILE),
                    )
                    kT_sb = kT_pool.tile([TILE, S], f16, tag="kT")
                    for t in range(NT):
                        dq = dq_pool.tile([TILE, TILE], f16, tag="dq")
                        nc.vector.tensor_copy(dq[:], ki8[:, t * TILE : (t + 1) * TILE])
                        tp = t_psum.tile([TILE, TILE], f16, tag="tp")
                        nc.tensor.transpose(tp[:], dq[:], ident[:])
                        nc.vector.tensor_copy(kT_sb[:, t * TILE : (t + 1) * TILE], tp[:])
                    # ---- V: int8 natural, dequant in place ----
                    vi8 = kvi8_pool.tile([TILE, NT * TILE], i8, tag="vi8")
                    nc.gpsimd.dma_start(
                        out=vi8[:].rearrange("p (t d) -> p t d", d=TILE),
                        in_=comb_d[
                            :S, KOFF + kv * 2 * TILE + TILE : KOFF + (kv + 1) * 2 * TILE
                        ].rearrange("(t p) d -> p t d", p=TILE),
                    )
                    v_sb = v_pool.tile([TILE, NT * TILE], f16, tag="vv")
                    for t in range(NT):
                        c = HL * NT + KVL * NT + kv * NT + t
                        nc.vector.tensor_scalar_mul(
                            v_sb[:, t * TILE : (t + 1) * TILE],
                            vi8[:, t * TILE : (t + 1) * TILE],
                            aux_f32(c, c + 1),
                        )
                    v_by_kv[kv] = v_sb

                # ---- Q: int8 natural -> dequant (per-token scale) -> qT ----
                qi8 = qi8_pool.tile([TILE, NT * TILE], i8, tag="qi8")
                nc.sync.dma_start(
                    out=qi8[:].rearrange("p (t d) -> p t d", d=TILE),
                    in_=comb_d[:S, hl * TILE : (hl + 1) * TILE].rearrange(
                        "(t p) d -> p t d", p=TILE
                    ),
                )
                qT_sb = qT_pool.tile([TILE, S], f16, tag="qT")
                for t in range(NT):
                    dq = dq_pool.tile([TILE, TILE], f16, tag="dq")
                    nc.vector.tensor_scalar_mul(
                        dq[:],
                        qi8[:, t * TILE : (t + 1) * TILE],
                        aux_f32(hl * NT + t, hl * NT + t + 1),
                    )
                    tp = t_psum.tile([TILE, TILE], f16, tag="tp")
                    nc.tensor.transpose(tp[:], dq[:], ident[:])
                    nc.vector.tensor_copy(qT_sb[:, t * TILE : (t + 1) * TILE], tp[:])

                pT = pT_pool.tile([TILE, SUMW], f16, tag="pT")
                pT_by_hl[hl] = pT

                # ---- QK^T + exp (per-key scale via activation scale AP) ----
                for kj in range(NT):
                    w = _band_width(kj, S)
                    off = OFF[kj]
                    q0 = kj * TILE
                    s_ps = s_psum.tile([TILE, WMAX], f32, tag="s")
                    for c0, cw in _chunks(w):
                        nc.tensor.matmul(
                            s_ps[:, c0 : c0 + cw],
                            lhsT=kT_sb[:, kj * TILE : (kj + 1) * TILE],
                            rhs=qT_sb[:, q0 + c0 : q0 + c0 + cw],
                            start=True,
                            stop=True,
                        )
                    nc.scalar.activation(
                        pT[:, off : off + w],
                        s_ps[:, :w],
                        mybir.ActivationFunctionType.Exp,
                        scale=aux_f32(HL * NT + kv * NT + kj, HL * NT + kv * NT + kj + 1),
                    )
                    nc.vector.tensor_mul(
                        pT[:, off : off + TILE],
                        pT[:, off : off + TILE],
                        mask_diag[:],
                    )
                    if kj * TILE + WINDOW + TILE <= S:
                        nc.vector.tensor_mul(
                            pT[:, off + WINDOW : off + WINDOW + TILE],
                            pT[:, off + WINDOW : off + WINDOW + TILE],
                            mask_win[:],
                        )

            def pv_phase(hl):
                kv = hl // 4 if HL >= 4 else 0
                v_sb = v_by_kv[kv]
                pT = pT_by_hl.pop(hl)
                out_row = orow_pool.tile([TILE, S], f32, tag="orow")
                # ---- PV + denominator, per query span ----
                # od_ps: one PSUM bank; cols [0,SPAN) = O^T, [SPAN,2*SPAN) = D
                for sp in range(NSPAN):
                    lo, hi = sp * SPAN, (sp + 1) * SPAN
                    ktiles = []
                    for kj in range(NT):
                        w = _band_width(kj, S)
                        qlo = max(kj * TILE, lo)
                        qhi = min(kj * TILE + w, hi)
                        if qhi > qlo:
                            ktiles.append((kj, qlo, qhi))
                    # full-span writers first (uniform psum zero-region state)
                    ktiles.sort(key=lambda t: 0 if (t[1] == lo and t[2] == hi) else 1)
                    assert ktiles[0][1] == lo and ktiles[0][2] == hi, (S, sp)

                    od_ps = o_psum.tile([TILE, 2 * SPAN], f32, tag="od")
                    n = len(ktiles)
                    for i, (kj, qlo, qhi) in enumerate(ktiles):
                        rel_p = OFF[kj] + (qlo - kj * TILE)
                        rel_o = qlo - lo
                        ln = qhi - qlo
                        rhs = pT[:, rel_p : rel_p + ln]
                        nc.tensor.matmul(
                            od_ps[:, rel_o : rel_o + ln],
                            lhsT=v_sb[:, kj * TILE : (kj + 1) * TILE],
                            rhs=rhs,
                            start=(i == 0),
                            stop=False,
                        )
                        nc.tensor.matmul(
                            od_ps[:, SPAN + rel_o : SPAN + rel_o + ln],
                            lhsT=ones[:, :],
                            rhs=rhs,
                            start=False,
                            stop=(i == n - 1),
                        )

                    d_sb = d_pool.tile([TILE, SPAN], f32, tag="d_sb")
                    nc.vector.tensor_scalar_add(
                        d_sb[:], od_ps[:, SPAN : 2 * SPAN], esk[:, hl : hl + 1]
                    )
                    nc.vector.reciprocal(d_sb[:], d_sb[:])
                    nc.vector.tensor_mul(out_row[:, lo:hi], od_ps[:, :SPAN], d_sb[:])

                # ---- per-channel int8 quantization of the O^T row ----
                m = stat_pool.tile([TILE, 1], f32, tag="m")
                nc.vector.tensor_reduce(
                    out=m[:],
                    in_=out_row[:],
                    axis=mybir.AxisListType.X,
                    op=mybir.AluOpType.max,
                    apply_absolute_value=True,
                )
                nc.vector.tensor_scalar_max(m[:], m[:], 1e-20)
                r = stat_pool.tile([TILE, 1], f32, tag="r")
                nc.vector.reciprocal(r[:], m[:])
                nc.vector.tensor_scalar_mul(r[:], r[:], 127.0)
                oi8 = oi8_pool.tile([TILE, S], i8, tag="oi8")
                nc.vector.tensor_scalar_mul(oi8[:], out_row[:], r[:, 0:1])
                # out-DMA on SWDGE: keeps SP's FIFO free for the next
                # head's loads
                nc.gpsimd.dma_start(
                    out=o_d[hl * TILE : (hl + 1) * TILE, :S], in_=oi8[:]
                )
                nc.sync.dma_start(
                    out=o_d[hl * TILE : (hl + 1) * TILE, S : S + 4],
                    in_=m[:].bitcast(i8),
                )

            # software pipeline across heads: QK(hl+1) is emitted before
            # PV(hl) so PV never chases a just-issued exp
            qk_phase(0)
            for hl in range(1, HL):
                qk_phase(hl)
                pv_phase(hl - 1)
            pv_phase(HL - 1)
    # Bacc lowering (wait splitting, reg alloc) must run before serialization;
    # nothing on the PJRT path calls it for us.
    nc.finalize()
    return nc


def _get_nc(S, HL, KVL):
    key = (S, HL, KVL)
    if key not in _CACHE:
        _CACHE[key] = build_nc(S, HL, KVL)
    return _CACHE[key]


def _get_jits(B, S):
    key = (B, S)
    if key in _JIT:
        return _JIT[key]
    import jax
    import jax.numpy as jnp

    PB = 8 // B
    HL = NUM_HEADS // PB
    KVL = max(1, NUM_KV_HEADS // PB)
    assert PB * KVL == NUM_KV_HEADS, (B, PB, KVL)
    NT = S // TILE
    D = HEAD_DIM
    PW = HL * TILE + KVL * 2 * TILE
    AUXW = HL * NT + 2 * KVL * NT + HL

    def quant(x, nh):
        T = x.shape[0]
        xr = x.reshape(T, nh, D)
        s = jnp.maximum(jnp.max(jnp.abs(xr), axis=2) / 127.0, 1e-12)  # [T,nh]
        xi = jnp.clip(jnp.round(xr / s[:, :, None]), -127, 127).astype(jnp.int8)
        return xi, s

    def scales_rows(s, nh, mul):
        # [T, PB*nh] f32 -> [B, PB, TILE, nh*NT]; col h*NT+t at partition p
        # holds the scale of token t*TILE+p, local head h
        t = (s * mul).reshape(B, NT, TILE, PB, nh).transpose(0, 3, 2, 4, 1)
        return t.reshape(B, PB, TILE, nh * NT)

    def prep(q, k, v, sinks):
        qi, qs = quant(q, NUM_HEADS)  # [T,16,D] i8, [T,16] f32
        ki, ks = quant(k, NUM_KV_HEADS)
        vi, vs = quant(v, NUM_KV_HEADS)
        qrows = qi.reshape(B, S, PB, HL * D)
        kvrows = jnp.concatenate([ki, vi], axis=2).reshape(B, S, PB, KVL * 2 * D)
        tok = jnp.concatenate([qrows, kvrows], axis=3)  # [B,S,PB,PW]
        tok = tok.transpose(0, 2, 1, 3)  # [B,PB,S,PW]
        aux = jnp.concatenate(
            [
                scales_rows(qs, HL, 1.0),
                scales_rows(ks, KVL, SCALE),
                scales_rows(vs, KVL, 1.0),
                jnp.broadcast_to(
                    sinks.reshape(1, PB, 1, HL), (B, PB, TILE, HL)
                ).astype(jnp.float32),
            ],
            axis=3,
        )  # [B,PB,TILE,AUXW] f32
        aux8 = jax.lax.bitcast_convert_type(aux, jnp.int8).reshape(
            B, PB, TILE, AUXW * 4
        )
        aux8 = jnp.pad(aux8, ((0, 0), (0, 0), (0, 0), (0, PW - AUXW * 4)))
        comb = jnp.concatenate([tok, aux8], axis=2)  # [B,PB,S+TILE,PW]
        return comb.reshape(8 * (S + TILE), PW)

    def assemble(o_list, B_, S_, PB_, HL_):
        raw = jnp.stack(o_list)  # [8, HL*128, S+4] int8
        oi8 = raw[:, :, :S_]
        om = jax.lax.bitcast_convert_type(raw[:, :, S_ : S_ + 4], jnp.float32)
        om = om[:, :, None]  # [8, HL*128, 1] f32
        o = oi8.astype(jnp.float32) * (om / 127.0)  # [8, HL*128, S]
        o = o.reshape(B_, PB_, HL_ * HEAD_DIM, S_)
        # out[b*S + s_, p*HL*128 + c] = o[b, p, c, s_]
        out = o.transpose(0, 3, 1, 2).reshape(B_ * S_, NUM_HEADS * HEAD_DIM)
        return out

    jits = {
        "cpu": jax.local_devices(backend="cpu")[0],
        "prep": jax.jit(prep),
        "assemble": jax.jit(assemble, static_argnums=(1, 2, 3, 4)),
    }
    _JIT[key] = jits
    return jits


def kernel(q, k, v, sinks, batch, seqlen):
    import jax

    from concourse.bass_utils import run_bass_kernel_spmd

    q = np.asarray(q)
    k = np.asarray(k)
    v = np.asarray(v)
    sinks = np.asarray(sinks)
    B = int(batch)
    S = int(seqlen)
    assert 8 % B == 0, B
    PB = 8 // B  # head-parts per batch
    HL = NUM_HEADS // PB
    KVL = max(1, NUM_KV_HEADS // PB)

    nc = _get_nc(S, HL, KVL)
    jits = _get_jits(B, S)

    with jax.default_device(jits["cpu"]):
        comb = np.asarray(jits["prep"](q, k, v, sinks))

    rows = S + TILE
    in_maps = [{"comb": comb[c * rows : (c + 1) * rows]} for c in range(8)]

    res = run_bass_kernel_spmd(nc, in_maps, core_ids=list(range(8)))
    o_list = [res.results[c]["o"] for c in range(8)]
    with jax.default_device(jits["cpu"]):
        out = np.asarray(jits["assemble"](o_list, B, S, PB, HL))
    return out


# revision 9
# speedup vs baseline: 1.2116x; 1.2116x over previous
"""Varlen causal sliding-window attention with per-head sink logits, on 8 trn2 cores.

Wall time under the axon tunnel is transfer-bound (~20 ms/MiB each way, plus
~34 ms per tensor), so the wire format is ONE int8 tensor per core holding
int8-quantized q / k|v (per-(token,head) scales) plus a trailing row-block with
the f32 scales bitcast to bytes; the output is int8 with per-channel scales.
All dequant/requant runs on-device where compute is ~free (cost-model exec
~0.3 ms vs ~1.4 s of transfer per call).

Sharding: data-parallel over (batch, head-group). Each core gets one batch's
tokens and 16/PB contiguous q-heads (PB = 8//B parts per batch) plus the
matching kv-heads.

Device input (per core), comb [S+128, PW] int8 with PW = HL*128 + KVL*256:
  rows [0,S):   cols [0, HL*128) = q natural int8;
                cols [HL*128, PW) = per local kv head, 128 k-cols | 128 v-cols
  rows [S,S+128): bitcast f32 aux row-block, value columns
                qs [HL*NT] | sks [KVL*NT] | vs [KVL*NT] | sinks [HL]
                (aux[p, h*NT+t] = scale of token t*128+p, head h; sks
                premultiplied by softmax SCALE)
Device output:
  o [HL*128, S+4] int8 (O^T layout); the last 4 bytes of each row are the
  bitcast f32 per-channel absmax m (host dequant is o[:, :S] * m/127).

Device kernel per head:
  - load: int8 tiles DMA'd natural-layout; q dequants per-token via DVE
    tensor_scalar (int8 x [128,1] f32 -> f16) then PE-transposes per 128-block
    into qT; k converts int8->f16 (scales NOT applied) and PE-transposes into
    kT; v dequants in place (lhsT consumed natural).
  - QK^T per 128-key tile into PSUM; exp evicts PSUM->SBUF f16 probs with the
    per-key scale folded into the activation's per-partition scale AP
    (logit = qdeq . k_int, exp scale = SCALE*ks_key); triangular masks fix the
    two band edges.
  - PV + ones-matmul denominator per 256-query span accumulate O^T and D in
    one PSUM bank; DVE adds exp(sink), reciprocal, multiply into an f32 O^T
    row; after all spans: rowmax(|O|), quantize row to int8, DMA out.
"""

import sys

sys.path.insert(0, "/opt/trn_rl_repo")

import numpy as np

# run_bass_via_pjrt builds a fresh jax.jit per call, so without a persistent
# compilation cache every warm call re-runs bir_verify_and_optimise +
# generate_dve_tables (~0.5 s on this 1-CPU host). With the cache, calls
# after the first deserialize the executable instead.
import jax as _jax

_jax.config.update("jax_compilation_cache_dir", "/tmp/jax_comp_cache")
_jax.config.update("jax_persistent_cache_min_compile_time_secs", 0)
_jax.config.update("jax_persistent_cache_min_entry_size_bytes", -1)

NUM_HEADS = 16
NUM_KV_HEADS = 4
HEAD_DIM = 128
WINDOW = 1024
SCALE = 0.08838834764831845
TILE = 128

_CACHE = {}
_JIT = {}


def _band_width(kj, S):
    # keys in tile kj are visible to queries q with 0 <= q - k <= WINDOW
    # -> q in [kj*TILE, kj*TILE + WINDOW + TILE), clipped to S
    return min(S, kj * TILE + WINDOW + TILE) - kj * TILE


def _chunks(w):
    # split [0, w) at 512 boundaries (PSUM bank) for matmul outputs
    out = []
    c0 = 0
    while c0 < w:
        out.append((c0, min(512, w - c0)))
        c0 += 512
    return out


def build_nc(S, HL, KVL):
    import concourse.bacc as bacc
    import concourse.mybir as mybir
    from concourse.masks import make_identity, make_lower_triangular, make_upper_triangular
    from concourse.tile import TileContext

    f32 = mybir.dt.float32
    f16 = mybir.dt.float16
    i8 = mybir.dt.int8
    NT = S // TILE
    WMAX = min(S, WINDOW + TILE)
    SUMW = sum(_band_width(kj, S) for kj in range(NT))
    OFF = np.cumsum([0] + [_band_width(kj, S) for kj in range(NT)]).tolist()
    SPAN = 256
    NSPAN = S // SPAN
    PW = HL * TILE + KVL * 2 * TILE
    AUXW = HL * NT + 2 * KVL * NT + HL
    KOFF = HL * TILE  # comb col where the kv block starts

    nc = bacc.Bacc()
    comb_d = nc.dram_tensor("comb", [S + TILE, PW], i8, kind="ExternalInput")
    # last 4 int8 cols of each row hold the f32 per-channel absmax (bitcast)
    o_d = nc.dram_tensor("o", [HL * TILE, S + 4], i8, kind="ExternalOutput")

    with TileContext(nc) as tc:
        with (
            tc.tile_pool(name="const", bufs=1) as const_pool,
            tc.tile_pool(name="qi8", bufs=2) as qi8_pool,
            tc.tile_pool(name="kvi8", bufs=2) as kvi8_pool,
            tc.tile_pool(name="dq", bufs=3) as dq_pool,
            tc.tile_pool(name="qT", bufs=3) as qT_pool,
            tc.tile_pool(name="kT", bufs=2) as kT_pool,
            tc.tile_pool(name="vv", bufs=2) as v_pool,
            tc.tile_pool(name="pT", bufs=3) as pT_pool,
            tc.tile_pool(name="dsb", bufs=3) as d_pool,
            tc.tile_pool(name="orow", bufs=2) as orow_pool,
            tc.tile_pool(name="oi8", bufs=2) as oi8_pool,
            tc.tile_pool(name="stat", bufs=4) as stat_pool,
            tc.tile_pool(name="spsum", bufs=1, space="PSUM") as s_psum,
            tc.tile_pool(name="opsum", bufs=2, space="PSUM") as o_psum,
            tc.tile_pool(name="tpsum", bufs=2, space="PSUM") as t_psum,
        ):
            mask_diag = const_pool.tile([TILE, TILE], f16)  # valid: q >= k
            mask_win = const_pool.tile([TILE, TILE], f16)  # valid: q <= k
            make_upper_triangular(nc, mask_diag[:], val=1.0, diag=True)
            make_lower_triangular(nc, mask_win[:], val=1.0, diag=True)
            ones = const_pool.tile([TILE, TILE], f16)
            nc.vector.memset(ones[:], 1.0)
            ident = const_pool.tile([TILE, TILE], f16)
            make_identity(nc, ident[:])
            aux_i8 = const_pool.tile([TILE, AUXW * 4], i8)
            nc.sync.dma_start(out=aux_i8[:], in_=comb_d[S : S + TILE, : AUXW * 4])

            def aux_f32(a, b):
                # f32 view of aux value columns [a, b)
                return aux_i8[:, a * 4 : b * 4].bitcast(f32)

            esk = const_pool.tile([TILE, HL], f32)
            nc.scalar.activation(
                esk[:],
                aux_f32(HL * NT + 2 * KVL * NT, AUXW),
                mybir.ActivationFunctionType.Exp,
            )

            kT_sb = None
            v_by_kv = {}
            pT_by_hl = {}

            def qk_phase(hl):
                nonlocal kT_sb
                kv = hl // 4 if HL >= 4 else 0
                if hl % 4 == 0 or kT_sb is None:
                    # ---- K: int8 natural -> f16 (unscaled) -> kT via PE ----
                    ki8 = kvi8_pool.tile([TILE, NT * TILE], i8, tag="ki8")
                    nc.sync.dma_start(
                        out=ki8[:].rearrange("p (t d) -> p t d", d=TILE),
                        in_=comb_d[
                            :S, KOFF + kv * 2 * TILE : KOFF + kv * 2 * TILE + TILE
                        ].rearrange("(t p) d -> p t d", p=TILE),
                    )
                    kT_sb = kT_pool.tile([TILE, S], f16, tag="kT")
                    for t in range(NT):
                        dq = dq_pool.tile([TILE, TILE], f16, tag="dq")
                        nc.vector.tensor_copy(dq[:], ki8[:, t * TILE : (t + 1) * TILE])
                        tp = t_psum.tile([TILE, TILE], f16, tag="tp")
                        nc.tensor.transpose(tp[:], dq[:], ident[:])
                        nc.vector.tensor_copy(kT_sb[:, t * TILE : (t + 1) * TILE], tp[:])
                    # ---- V: int8 natural, dequant in place ----
                    vi8 = kvi8_pool.tile([TILE, NT * TILE], i8, tag="vi8")
                    nc.gpsimd.dma_start(
                        out=vi8[:].rearrange("p (t d) -> p t d", d=TILE),
                        in_=comb_d[
                            :S, KOFF + kv * 2 * TILE + TILE : KOFF + (kv + 1) * 2 * TILE
                        ].rearrange("(t p) d -> p t d", p=TILE),
                    )
                    v_sb = v_pool.tile([TILE, NT * TILE], f16, tag="vv")
                    for t in range(NT):
                        c = HL * NT + KVL * NT + kv * NT + t
                        nc.vector.tensor_scalar_mul(
                            v_sb[:, t * TILE : (t + 1) * TILE],
                            vi8[:, t * TILE : (t + 1) * TILE],
                            aux_f32(c, c + 1),
                        )
                    v_by_kv[kv] = v_sb

                # ---- Q: int8 natural -> dequant (per-token scale) -> qT ----
                qi8 = qi8_pool.tile([TILE, NT * TILE], i8, tag="qi8")
                nc.sync.dma_start(
                    out=qi8[:].rearrange("p (t d) -> p t d", d=TILE),
                    in_=comb_d[:S, hl * TILE : (hl + 1) * TILE].rearrange(
                        "(t p) d -> p t d", p=TILE
                    ),
                )
                qT_sb = qT_pool.tile([TILE, S], f16, tag="qT")
                for t in range(NT):
                    dq = dq_pool.tile([TILE, TILE], f16, tag="dq")
                    nc.vector.tensor_scalar_mul(
                        dq[:],
                        qi8[:, t * TILE : (t + 1) * TILE],
                        aux_f32(hl * NT + t, hl * NT + t + 1),
                    )
                    tp = t_psum.tile([TILE, TILE], f16, tag="tp")
                    nc.tensor.transpose(tp[:], dq[:], ident[:])
                    nc.vector.tensor_copy(qT_sb[:, t * TILE : (t + 1) * TILE], tp[:])

                pT = pT_pool.tile([TILE, SUMW], f16, tag="pT")
                pT_by_hl[hl] = pT

                # ---- QK^T + exp (per-key scale via activation scale AP) ----
                for kj in range(NT):
                    w = _band_width(kj, S)
                    off = OFF[kj]
                    q0 = kj * TILE
                    s_ps = s_psum.tile([TILE, WMAX], f32, tag="s")
                    for c0, cw in _chunks(w):
                        nc.tensor.matmul(
                            s_ps[:, c0 : c0 + cw],
                            lhsT=kT_sb[:, kj * TILE : (kj + 1) * TILE],
                            rhs=qT_sb[:, q0 + c0 : q0 + c0 + cw],
                            start=True,
                            stop=True,
                        )
                    nc.scalar.activation(
                        pT[:, off : off + w],
                        s_ps[:, :w],
                        mybir.ActivationFunctionType.Exp,
                        scale=aux_f32(HL * NT + kv * NT + kj, HL * NT + kv * NT + kj + 1),
                    )
                    nc.vector.tensor_mul(
                        pT[:, off : off + TILE],
                        pT[:, off : off + TILE],
                        mask_diag[:],
                    )
                    if kj * TILE + WINDOW + TILE <= S:
                        nc.vector.tensor_mul(
                            pT[:, off + WINDOW : off + WINDOW + TILE],
                            pT[:, off + WINDOW : off + WINDOW + TILE],
                            mask_win[:],
                        )

            def pv_phase(hl):
                kv = hl // 4 if HL >= 4 else 0
                v_sb = v_by_kv[kv]
                pT = pT_by_hl.pop(hl)
                out_row = orow_pool.tile([TILE, S], f32, tag="orow")
                # ---- PV + denominator, per query span ----
                # od_ps: one PSUM bank; cols [0,SPAN) = O^T, [SPAN,2*SPAN) = D
                for sp in range(NSPAN):
                    lo, hi = sp * SPAN, (sp + 1) * SPAN
                    ktiles = []
                    for kj in range(NT):
                        w = _band_width(kj, S)
                        qlo = max(kj * TILE, lo)
                        qhi = min(kj * TILE + w, hi)
                        if qhi > qlo:
                            ktiles.append((kj, qlo, qhi))
                    # full-span writers first (uniform psum zero-region state)
                    ktiles.sort(key=lambda t: 0 if (t[1] == lo and t[2] == hi) else 1)
                    assert ktiles[0][1] == lo and ktiles[0][2] == hi, (S, sp)

                    od_ps = o_psum.tile([TILE, 2 * SPAN], f32, tag="od")
                    n = len(ktiles)
                    for i, (kj, qlo, qhi) in enumerate(ktiles):
                        rel_p = OFF[kj] + (qlo - kj * TILE)
                        rel_o = qlo - lo
                        ln = qhi - qlo
                        rhs = pT[:, rel_p : rel_p + ln]
                        nc.tensor.matmul(
                            od_ps[:, rel_o : rel_o + ln],
                            lhsT=v_sb[:, kj * TILE : (kj + 1) * TILE],
                            rhs=rhs,
                            start=(i == 0),
                            stop=False,
                        )
                        nc.tensor.matmul(
                            od_ps[:, SPAN + rel_o : SPAN + rel_o + ln],
                            lhsT=ones[:, :],
                            rhs=rhs,
                            start=False,
                            stop=(i == n - 1),
                        )

                    d_sb = d_pool.tile([TILE, SPAN], f32, tag="d_sb")
                    nc.vector.tensor_scalar_add(
                        d_sb[:], od_ps[:, SPAN : 2 * SPAN], esk[:, hl : hl + 1]
                    )
                    nc.vector.reciprocal(d_sb[:], d_sb[:])
                    nc.vector.tensor_mul(out_row[:, lo:hi], od_ps[:, :SPAN], d_sb[:])

                # ---- per-channel int8 quantization of the O^T row ----
                m = stat_pool.tile([TILE, 1], f32, tag="m")
                nc.vector.tensor_reduce(
                    out=m[:],
                    in_=out_row[:],
                    axis=mybir.AxisListType.X,
                    op=mybir.AluOpType.max,
                    apply_absolute_value=True,
                )
                nc.vector.tensor_scalar_max(m[:], m[:], 1e-20)
                r = stat_pool.tile([TILE, 1], f32, tag="r")
                nc.vector.reciprocal(r[:], m[:])
                nc.vector.tensor_scalar_mul(r[:], r[:], 127.0)
                oi8 = oi8_pool.tile([TILE, S], i8, tag="oi8")
                nc.vector.tensor_scalar_mul(oi8[:], out_row[:], r[:, 0:1])
                # out-DMA on SWDGE: keeps SP's FIFO free for the next
                # head's loads
                nc.gpsimd.dma_start(
                    out=o_d[hl * TILE : (hl + 1) * TILE, :S], in_=oi8[:]
                )
                nc.sync.dma_start(
                    out=o_d[hl * TILE : (hl + 1) * TILE, S : S + 4],
                    in_=m[:].bitcast(i8),
                )

            # software pipeline across heads: QK(hl+1) is emitted before
            # PV(hl) so PV never chases a just-issued exp
            qk_phase(0)
            for hl in range(1, HL):
                qk_phase(hl)
                pv_phase(hl - 1)
            pv_phase(HL - 1)
    # Bacc lowering (wait splitting, reg alloc) must run before serialization;
    # nothing on the PJRT path calls it for us.
    nc.finalize()
    return nc


def _get_nc(S, HL, KVL):
    key = (S, HL, KVL)
    if key not in _CACHE:
        _CACHE[key] = build_nc(S, HL, KVL)
    return _CACHE[key]


def _get_jits(B, S):
    key = (B, S)
    if key in _JIT:
        return _JIT[key]
    import jax
    import jax.numpy as jnp

    PB = 8 // B
    HL = NUM_HEADS // PB
    KVL = max(1, NUM_KV_HEADS // PB)
    assert PB * KVL == NUM_KV_HEADS, (B, PB, KVL)
    NT = S // TILE
    D = HEAD_DIM
    PW = HL * TILE + KVL * 2 * TILE
    AUXW = HL * NT + 2 * KVL * NT + HL

    def quant(x, nh):
        T = x.shape[0]
        xr = x.reshape(T, nh, D)
        s = jnp.maximum(jnp.max(jnp.abs(xr), axis=2) / 127.0, 1e-12)  # [T,nh]
        xi = jnp.clip(jnp.round(xr / s[:, :, None]), -127, 127).astype(jnp.int8)
        return xi, s

    def scales_rows(s, nh, mul):
        # [T, PB*nh] f32 -> [B, PB, TILE, nh*NT]; col h*NT+t at partition p
        # holds the scale of token t*TILE+p, local head h
        t = (s * mul).reshape(B, NT, TILE, PB, nh).transpose(0, 3, 2, 4, 1)
        return t.reshape(B, PB, TILE, nh * NT)

    def prep(q, k, v, sinks):
        qi, qs = quant(q, NUM_HEADS)  # [T,16,D] i8, [T,16] f32
        ki, ks = quant(k, NUM_KV_HEADS)
        vi, vs = quant(v, NUM_KV_HEADS)
        qrows = qi.reshape(B, S, PB, HL * D)
        kvrows = jnp.concatenate([ki, vi], axis=2).reshape(B, S, PB, KVL * 2 * D)
        tok = jnp.concatenate([qrows, kvrows], axis=3)  # [B,S,PB,PW]
        tok = tok.transpose(0, 2, 1, 3)  # [B,PB,S,PW]
        aux = jnp.concatenate(
            [
                scales_rows(qs, HL, 1.0),
                scales_rows(ks, KVL, SCALE),
                scales_rows(vs, KVL, 1.0),
                jnp.broadcast_to(
                    sinks.reshape(1, PB, 1, HL), (B, PB, TILE, HL)
                ).astype(jnp.float32),
            ],
            axis=3,
        )  # [B,PB,TILE,AUXW] f32
        aux8 = jax.lax.bitcast_convert_type(aux, jnp.int8).reshape(
            B, PB, TILE, AUXW * 4
        )
        aux8 = jnp.pad(aux8, ((0, 0), (0, 0), (0, 0), (0, PW - AUXW * 4)))
        comb = jnp.concatenate([tok, aux8], axis=2)  # [B,PB,S+TILE,PW]
        return comb.reshape(8 * (S + TILE), PW)

    def assemble(o_list, B_, S_, PB_, HL_):
        raw = jnp.stack(o_list)  # [8, HL*128, S+4] int8
        oi8 = raw[:, :, :S_]
        om = jax.lax.bitcast_convert_type(raw[:, :, S_ : S_ + 4], jnp.float32)
        om = om[:, :, None]  # [8, HL*128, 1] f32
        o = oi8.astype(jnp.float32) * (om / 127.0)  # [8, HL*128, S]
        o = o.reshape(B_, PB_, HL_ * HEAD_DIM, S_)
        # out[b*S + s_, p*HL*128 + c] = o[b, p, c, s_]
        out = o.transpose(0, 3, 1, 2).reshape(B_ * S_, NUM_HEADS * HEAD_DIM)
        return out

    jits = {
        "cpu": jax.local_devices(backend="cpu")[0],
        "prep": jax.jit(prep),
        "assemble": jax.jit(assemble, static_argnums=(1, 2, 3, 4)),
    }
    _JIT[key] = jits
    return jits


def kernel(q, k, v, sinks, batch, seqlen):
    import jax

    from concourse.bass_utils import run_bass_kernel_spmd

    q = np.asarray(q)
    k = np.asarray(k)
    v = np.asarray(v)
    sinks = np.asarray(sinks)
    B = int(batch)
    S = int(seqlen)
    assert 8 % B == 0, B
    PB = 8 // B  # head-parts per batch
    HL = NUM_HEADS // PB
    KVL = max(1, NUM_KV_HEADS // PB)

    nc = _get_nc(S, HL, KVL)
    jits = _get_jits(B, S)

    with jax.default_device(jits["cpu"]):
        comb = np.asarray(jits["prep"](q, k, v, sinks))

    rows = S + TILE
    in_maps = [{"comb": comb[c * rows : (c + 1) * rows]} for c in range(8)]

    res = run_bass_kernel_spmd(nc, in_maps, core_ids=list(range(8)))
    o_list = [res.results[c]["o"] for c in range(8)]
    with jax.default_device(jits["cpu"]):
        out = np.asarray(jits["assemble"](o_list, B, S, PB, HL))
    return out
